# revision 62
# baseline (speedup 1.0000x reference)
# Bahdanau attention kernel for Trainium2 (Bass/Tile), 8-core data-parallel.
#
# Problem (per reference):
#   raw[b,t,e] = sum_d x[b,t,d] w[e,d] + ub[b,e],  ub = query @ u.T
#   score[b,t] = sum_e tanh(raw[b,t,e]) v[e]   (+ -inf mask for t >= lengths[b])
#   weights    = softmax(score over t)
#   expect[b,d]= sum_t weights[b,t] x[b,t,d]
#
# Sharding: batch (32) split 4-per-core across 8 cores; w/u/v replicated.
# Device layout ("Layout B"): rawT[e,t] accumulated in PSUM with d on
# partitions for both matmul operands; x is transposed on-chip with one big
# xbar DMA-transpose per batch (fp16). The ub bias is folded into the Tanh
# activation's per-partition bias. score = v.T @ tanh(rawT) via PE matmuls.
# Per-batch mask/softmax/expectation overlap the next batch's matmuls.
import numpy as np

B, T, D, QD = 32, 2048, 512, 512
NCORES = 8
BL = B // NCORES  # 4 batches per core

_CACHE = {}


def _build_nc(repeat=1):
    from contextlib import ExitStack

    import concourse.bacc as bacc
    import concourse.mybir as mybir
    import concourse.tile as tile

    F32 = mybir.dt.float32
    F16 = mybir.dt.float16
    I32 = mybir.dt.int32
    AF = mybir.ActivationFunctionType
    ALU = mybir.AluOpType
    AX = mybir.AxisListType

    nc = bacc.Bacc("TRN2", target_bir_lowering=False, debug=False)

    x = nc.dram_tensor("x", [BL, T, D], F32, kind="ExternalInput").ap()
    wt = nc.dram_tensor("wt", [128, 4, 512], F16, kind="ExternalInput").ap()
    ubt = nc.dram_tensor("ubt", [128, 4, BL], F32, kind="ExternalInput").ap()
    vt = nc.dram_tensor("vt", [128, 4], F16, kind="ExternalInput").ap()
    lc = nc.dram_tensor("lc", [BL, 1], F32, kind="ExternalInput").ap()
    score_o = nc.dram_tensor("score_o", [BL, T], F32, kind="ExternalOutput").ap()
    weights_o = nc.dram_tensor("weights_o", [BL, T], F32, kind="ExternalOutput").ap()
    expect_o = nc.dram_tensor("expect_o", [BL, D], F32, kind="ExternalOutput").ap()

    with tile.TileContext(nc) as tc, ExitStack() as ctx:
        const = ctx.enter_context(tc.tile_pool(name="const", bufs=1))
        xbp = ctx.enter_context(tc.tile_pool(name="xbp", bufs=1))
        xtp = ctx.enter_context(tc.tile_pool(name="xtp", bufs=2))
        thp = ctx.enter_context(tc.tile_pool(name="thp", bufs=12))
        rowp = ctx.enter_context(tc.tile_pool(name="rowp", bufs=1))
        praw = ctx.enter_context(tc.tile_pool(name="praw", bufs=5, space="PSUM"))
        pvec = ctx.enter_context(tc.tile_pool(name="pvec", bufs=2, space="PSUM"))

        # ---- constants ----
        wt_sb = const.tile([128, 4, 512], F16, tag="wt")
        nc.sync.dma_start(wt_sb[:], wt)
        ub_sb = const.tile([128, 4, BL], F32, tag="ub")
        nc.sync.dma_start(ub_sb[:], ubt)
        v_sb = const.tile([128, 4], F16, tag="vt")
        nc.sync.dma_start(v_sb[:], vt)
        lc_sb = const.tile([BL, 1], F32, tag="lc")
        nc.sync.dma_start(lc_sb[:], lc)

        # mask prep tiles (emitted lazily, after the first loads, so the
        # iota doesn't block the cast-loads on the Pool sequencer)
        iota_i = const.tile([BL, T], F32, tag="ioi")
        inval4 = const.tile([BL, T], mybir.dt.int8, tag="inval4")
        ninf_row = const.tile([1, T], F32, tag="ninfr")

        def mask_prep():
            nc.gpsimd.iota(
                iota_i[:], pattern=[[1, T]], base=0, channel_multiplier=0,
                allow_small_or_imprecise_dtypes=True,
            )
            nc.vector.tensor_scalar(
                inval4[:], iota_i[:], lc_sb[:], None, op0=ALU.is_ge
            )
            nc.vector.memset(ninf_row[:], float("-inf"))

        xb = xbp.tile([128, BL, 16, 512], F16, tag="xb")
        state = {"prev_tr": None}

        def batch_head(b):
            """load + transpose + raw matmuls + tanh + score for batch b"""
            srow = rowp.tile([1, T], F32, name=f"srow{b}", tag=f"srow{b}")

            def score_mms(t, ths_t):
                ps = pvec.tile([1, 512], F32, name="ps", tag="pv")
                for e in range(4):
                    nc.tensor.matmul(
                        ps[:], lhsT=v_sb[:, e:e + 1], rhs=ths_t[e][:],
                        start=(e == 0), stop=(e == 3),
                    )
                # unscramble: psum col (nl, tpo) -> srow t = tpo*16 + 4t + nl
                nc.vector.tensor_copy(
                    srow[0:1, :].rearrange("one (tp g) -> one g tp", g=16)[
                        :, 4 * t:4 * t + 4, :
                    ],
                    ps[:].rearrange("one (nl tp) -> one nl tp", nl=4),
                )

            # HBM fp32 -> SBUF fp16 cast load (SWDGE). Partition p holds rows
            # t in [16p, 16p+16) => contiguous per-partition descriptors.
            # Batch 0 is split in halves so the first matmuls start sooner.
            xtb = xtp.tile([128, 64, 128], F16, tag="xt")
            xnat = x[b].rearrange("(p n) d -> p n d", n=16)
            segs = [(0, 8), (8, 16)] if b == 0 else [(0, 16)]
            for n0, n1 in segs:
                ld = nc.gpsimd.dma_start(xb[:, b, n0:n1, :], xnat[:, n0:n1, :])
                if state.get("prev_tr") is not None:
                    # keep the DMA engines working on the critical-path
                    # transpose before prefetching the next load
                    tile.add_dep_helper(
                        ld.ins, state["prev_tr"].ins, sync=True,
                        reason="load after previous transpose: DMA order",
                    )
                # xbar transpose: in [tp=128p, f=(n d)] -> out[dl, n*4+dc, tp]
                # holding x[t = tp*16 + n, d = dc*128 + dl]
                state["prev_tr"] = nc.sync.dma_start_transpose(
                    xtb[:, 4 * n0:4 * n1, :], xb[:, b, n0:n1, :]
                )
            xt16 = xtb[:].rearrange("p (q nl c) j -> p q nl c j", nl=4, c=4)

            pend = None  # (t, ths) with score matmuls not yet emitted
            for t in range(4):  # quarter q: covers t = tp*16 + 4q + nl
                xt4 = xt16[:, t]
                ths = []
                for e in range(4):  # e-chunks of 128
                    pr = praw.tile([128, 512], F32, tag="praw")
                    for dc in range(4):  # contraction chunks of 128
                        # psum col = nl*128 + tp  <->  t = tp*16 + 4t + nl
                        nc.tensor.matmul(
                            pr[:],
                            lhsT=wt_sb[:, dc, e * 128:(e + 1) * 128],
                            rhs=xt4[:, :, dc, :],
                            start=(dc == 0),
                            stop=(dc == 3),
                        )
                    th = thp.tile([128, 512], F16, tag="th")
                    nc.scalar.activation(
                        th[:], pr[:], AF.Tanh, bias=ub_sb[:, e, b:b + 1]
                    )
                    ths.append(th)
                # defer score matmuls one chunk so PE never waits on tanh
                if pend is not None:
                    score_mms(*pend)
                pend = (t, ths)
            score_mms(*pend)
            return srow

        mrows = {}

        def prefetch_masks():
            for b in range(BL):
                mrow = rowp.tile(
                    [1, T], mybir.dt.int8, name=f"mrow{b}", tag=f"mrow{b}"
                )
                nc.sync.dma_start(mrow[:], inval4[b:b + 1, :])
                mrows[b] = mrow

        def tail_pre(b, srow):
            """mask, outputs, softmax, exp-weight transpose (no PE MMs)"""
            nc.vector.copy_predicated(srow[:], mrows[b][:], ninf_row[:])
            nc.sync.dma_start(score_o[b:b + 1, :], srow[:])

            # masked max (valid positions only) for a stable softmax shift
            mx = rowp.tile([1, 1], F32, tag="mx")
            nc.vector.reduce_max(mx[:], srow[:], axis=AX.X)
            nmx = rowp.tile([1, 1], F32, tag="nmx")
            nc.vector.tensor_scalar_mul(nmx[:], mx[:], -1.0)

            # softmax on [1, T]; sum folded into the Exp via accum_out
            ew = rowp.tile([1, T], F32, tag="ew", bufs=2)
            sm = rowp.tile([1, 1], F32, tag="sm")
            nc.scalar.activation(
                ew[:], srow[:], AF.Exp, bias=nmx[:], accum_out=sm[:]
            )
            # unnormalized-exp transpose via one DMA: wcol[p, n] = ew[p*16+n];
            # the 1/sum scale is folded into the expectation epilogue
            wcol = rowp.tile([128, 16], F32, name=f"wcol{b}", tag=f"wcol{b}")
            nc.sync.dma_start(wcol[:], ew[:])
            wcol_h = rowp.tile([128, 16], F16, name=f"wcolh{b}", tag=f"wcolh{b}")
            nc.vector.tensor_copy(wcol_h[:], wcol[:])

            rc = rowp.tile([1, 1], F32, name=f"rc{b}", tag=f"rc{b}")
            nc.vector.reciprocal(rc[:], sm[:])
            # normalized weights overwrite srow's slot (srow dead after Exp)
            wgt_row = srow
            nc.vector.tensor_scalar(wgt_row[:], ew[:], rc[:], None, op0=ALU.mult)
            nc.sync.dma_start(weights_o[b:b + 1, :], wgt_row[:])
            return wcol_h, rc

        def tail_pe(b, wcol_h, rc):
            """expectation: expect[b,:] = rc * sum_n wcol[:,n].T @ xb[:,b,n,:]"""
            pe_t = pvec.tile([1, 512], F32, tag="pv")
            for n in range(16):
                nc.tensor.matmul(
                    pe_t[:], lhsT=wcol_h[:, n:n + 1], rhs=xb[:, b, n, :],
                    start=(n == 0), stop=(n == 15),
                )
            erow = rowp.tile([1, D], F32, tag="erow", bufs=2)
            nc.vector.tensor_scalar(erow[:], pe_t[:], rc[:], None, op0=ALU.mult)
            nc.sync.dma_start(expect_o[b:b + 1, :], erow[:])

        # repeat>1 re-runs the body for steady-state HW timing (outputs are
        # overwritten identically each time). All heads emitted before all
        # tails so the PE stream never stalls on a softmax chain.
        for rep in range(repeat):
            srows = {}
            for b in range(BL):
                srows[b] = batch_head(b)
                if rep == 0 and b == 0:
                    mask_prep()
            if rep == 0:
                prefetch_masks()
            # batch 3's chain first: its srow is the last produced, so its
            # tail latency hides behind the other batches' tails
            order = [BL - 1] + list(range(BL - 1))
            wcols = {}
            for b in order:
                wcols[b] = tail_pre(b, srows[b])
            for b in order:
                tail_pe(b, *wcols[b])

    nc.compile()
    return nc


def _host_prep(w, u, v, query, lengths):
    """Host-side prep of the small replicated operands (layout + dtype)."""
    # wt_sb[p, dc, e] = w[e, dc*128+p]
    wt = np.ascontiguousarray(
        w.T.reshape(4, 128, D).transpose(1, 0, 2)
    ).astype(np.float16)
    # ub[b, e] = sum_q query[b, q] * u[e, q]
    ub = (query.astype(np.float64) @ u.T.astype(np.float64)).astype(np.float32)
    # vt[p, ec] = v[ec*128+p]
    vt = np.ascontiguousarray(v[:, 0].reshape(4, 128).T).astype(np.float16)
    lenf = lengths.astype(np.float32).reshape(B, 1)
    return wt, ub, vt, lenf


def make_in_maps(output_data, query, lengths, w, u, v):
    wt, ub, vt, lenf = _host_prep(w, u, v, query, lengths)
    in_maps = []
    for c in range(NCORES):
        sl = slice(c * BL, (c + 1) * BL)
        ub_s = np.ascontiguousarray(
            ub[sl].reshape(BL, 4, 128).transpose(2, 1, 0)
        )
        in_maps.append({
            "x": np.ascontiguousarray(output_data[sl]),
            "wt": wt,
            "ubt": ub_s,
            "vt": vt,
            "lc": np.ascontiguousarray(lenf[sl]),
        })
    return in_maps


def kernel(output_data, query, lengths, w, u, v):
    from concourse import bass_utils

    if "nc" not in _CACHE:
        _CACHE["nc"] = _build_nc()
    nc = _CACHE["nc"]

    in_maps = make_in_maps(output_data, query, lengths, w, u, v)
    res = bass_utils.run_bass_kernel_spmd(
        nc, in_maps, core_ids=list(range(NCORES))
    )
    score = np.concatenate(
        [res.results[c]["score_o"] for c in range(NCORES)], axis=0
    ).reshape(B, T, 1)
    weights = np.concatenate(
        [res.results[c]["weights_o"] for c in range(NCORES)], axis=0
    ).reshape(B, T, 1)
    expectation = np.concatenate(
        [res.results[c]["expect_o"] for c in range(NCORES)], axis=0
    )
    return score, weights, expectation


# revision 63
# speedup vs baseline: 1.1869x; 1.1869x over previous
# Bahdanau attention kernel for Trainium2 (Bass/Tile), 8-core data-parallel.
#
# Problem (per reference):
#   raw[b,t,e] = sum_d x[b,t,d] w[e,d] + ub[b,e],  ub = query @ u.T
#   score[b,t] = sum_e tanh(raw[b,t,e]) v[e]   (+ -inf mask for t >= lengths[b])
#   weights    = softmax(score over t)
#   expect[b,d]= sum_t weights[b,t] x[b,t,d]
#
# Sharding: batch (32) split 4-per-core across 8 cores; w/u/v replicated.
# Device layout ("Layout B"): rawT[e,t] accumulated in PSUM with d on
# partitions for both matmul operands; x is transposed on-chip with one big
# xbar DMA-transpose per batch (fp16). The ub bias is folded into the Tanh
# activation's per-partition bias. score = v.T @ tanh(rawT) via PE matmuls.
# Per-batch mask/softmax/expectation overlap the next batch's matmuls.
import numpy as np

B, T, D, QD = 32, 2048, 512, 512
NCORES = 8
BL = B // NCORES  # 4 batches per core

_CACHE = {}


def _build_nc(repeat=1):
    from contextlib import ExitStack

    import concourse.bacc as bacc
    import concourse.mybir as mybir
    import concourse.tile as tile

    F32 = mybir.dt.float32
    F16 = mybir.dt.float16
    I32 = mybir.dt.int32
    AF = mybir.ActivationFunctionType
    ALU = mybir.AluOpType
    AX = mybir.AxisListType

    nc = bacc.Bacc("TRN2", target_bir_lowering=False, debug=False)

    x = nc.dram_tensor("x", [BL, T, D], F32, kind="ExternalInput").ap()
    wt = nc.dram_tensor("wt", [128, 4, 512], F16, kind="ExternalInput").ap()
    ubt = nc.dram_tensor("ubt", [128, 4, BL], F32, kind="ExternalInput").ap()
    vt = nc.dram_tensor("vt", [128, 4], F16, kind="ExternalInput").ap()
    lc = nc.dram_tensor("lc", [BL, 1], F32, kind="ExternalInput").ap()
    score_o = nc.dram_tensor("score_o", [BL, T], F32, kind="ExternalOutput").ap()
    weights_o = nc.dram_tensor("weights_o", [BL, T], F32, kind="ExternalOutput").ap()
    expect_o = nc.dram_tensor("expect_o", [BL, D], F32, kind="ExternalOutput").ap()

    with tile.TileContext(nc) as tc, ExitStack() as ctx:
        const = ctx.enter_context(tc.tile_pool(name="const", bufs=1))
        xbp = ctx.enter_context(tc.tile_pool(name="xbp", bufs=1))
        xtp = ctx.enter_context(tc.tile_pool(name="xtp", bufs=2))
        thp = ctx.enter_context(tc.tile_pool(name="thp", bufs=12))
        rowp = ctx.enter_context(tc.tile_pool(name="rowp", bufs=1))
        praw = ctx.enter_context(tc.tile_pool(name="praw", bufs=5, space="PSUM"))
        pvec = ctx.enter_context(tc.tile_pool(name="pvec", bufs=2, space="PSUM"))

        # ---- constants ----
        wt_sb = const.tile([128, 4, 512], F16, tag="wt")
        nc.sync.dma_start(wt_sb[:], wt)
        ub_sb = const.tile([128, 4, BL], F32, tag="ub")
        nc.sync.dma_start(ub_sb[:], ubt)
        v_sb = const.tile([128, 4], F16, tag="vt")
        nc.sync.dma_start(v_sb[:], vt)
        lc_sb = const.tile([BL, 1], F32, tag="lc")
        nc.sync.dma_start(lc_sb[:], lc)

        # mask prep tiles (emitted lazily, after the first loads, so the
        # iota doesn't block the cast-loads on the Pool sequencer)
        iota_i = const.tile([BL, T], F32, tag="ioi")
        inval4 = const.tile([BL, T], mybir.dt.int8, tag="inval4")
        ninf_row = const.tile([1, T], F32, tag="ninfr")

        def mask_prep():
            nc.gpsimd.iota(
                iota_i[:], pattern=[[1, T]], base=0, channel_multiplier=0,
                allow_small_or_imprecise_dtypes=True,
            )
            nc.vector.tensor_scalar(
                inval4[:], iota_i[:], lc_sb[:], None, op0=ALU.is_ge
            )
            nc.vector.memset(ninf_row[:], float("-inf"))

        xb = xbp.tile([128, BL, 16, 512], F16, tag="xb")
        state = {"prev_tr": None}

        def batch_head(b):
            """load + transpose + raw matmuls + tanh + score for batch b"""
            srow = rowp.tile([1, T], F32, name=f"srow{b}", tag=f"srow{b}")

            def score_mms(t, ths_t):
                ps = pvec.tile([1, 512], F32, name="ps", tag="pv")
                for e in range(4):
                    nc.tensor.matmul(
                        ps[:], lhsT=v_sb[:, e:e + 1], rhs=ths_t[e][:],
                        start=(e == 0), stop=(e == 3),
                    )
                # unscramble: psum col (nl, tpo) -> srow t = tpo*16 + 4t + nl
                nc.vector.tensor_copy(
                    srow[0:1, :].rearrange("one (tp g) -> one g tp", g=16)[
                        :, 4 * t:4 * t + 4, :
                    ],
                    ps[:].rearrange("one (nl tp) -> one nl tp", nl=4),
                )

            # HBM fp32 -> SBUF fp16 cast load (SWDGE). Partition p holds rows
            # t in [16p, 16p+16) => contiguous per-partition descriptors.
            # Batch 0 is split in halves so the first matmuls start sooner.
            xtb = xtp.tile([128, 64, 128], F16, tag="xt")
            xnat = x[b].rearrange("(p n) d -> p n d", n=16)
            segs = [(0, 8), (8, 16)] if b == 0 else [(0, 16)]
            for n0, n1 in segs:
                ld = nc.gpsimd.dma_start(xb[:, b, n0:n1, :], xnat[:, n0:n1, :])
                if state.get("prev_tr") is not None:
                    # keep the DMA engines working on the critical-path
                    # transpose before prefetching the next load
                    tile.add_dep_helper(
                        ld.ins, state["prev_tr"].ins, sync=True,
                        reason="load after previous transpose: DMA order",
                    )
                # xbar transpose: in [tp=128p, f=(n d)] -> out[dl, n*4+dc, tp]
                # holding x[t = tp*16 + n, d = dc*128 + dl]
                state["prev_tr"] = nc.sync.dma_start_transpose(
                    xtb[:, 4 * n0:4 * n1, :], xb[:, b, n0:n1, :]
                )
            xt16 = xtb[:].rearrange("p (q nl c) j -> p q nl c j", nl=4, c=4)

            pend = None  # (t, ths) with score matmuls not yet emitted
            for t in range(4):  # quarter q: covers t = tp*16 + 4q + nl
                xt4 = xt16[:, t]
                ths = []
                for e in range(4):  # e-chunks of 128
                    pr = praw.tile([128, 512], F32, tag="praw")
                    for dc in range(4):  # contraction chunks of 128
                        # psum col = nl*128 + tp  <->  t = tp*16 + 4t + nl
                        nc.tensor.matmul(
                            pr[:],
                            lhsT=wt_sb[:, dc, e * 128:(e + 1) * 128],
                            rhs=xt4[:, :, dc, :],
                            start=(dc == 0),
                            stop=(dc == 3),
                        )
                    th = thp.tile([128, 512], F16, tag="th")
                    nc.scalar.activation(
                        th[:], pr[:], AF.Tanh, bias=ub_sb[:, e, b:b + 1]
                    )
                    ths.append(th)
                # defer score matmuls one chunk so PE never waits on tanh
                if pend is not None:
                    score_mms(*pend)
                pend = (t, ths)
            score_mms(*pend)
            return srow

        mrows = {}

        def prefetch_masks():
            for b in range(BL):
                mrow = rowp.tile(
                    [1, T], mybir.dt.int8, name=f"mrow{b}", tag=f"mrow{b}"
                )
                nc.sync.dma_start(mrow[:], inval4[b:b + 1, :])
                mrows[b] = mrow

        def tail_pre(b, srow):
            """mask, outputs, softmax, exp-weight transpose (no PE MMs)"""
            nc.vector.copy_predicated(srow[:], mrows[b][:], ninf_row[:])
            nc.sync.dma_start(score_o[b:b + 1, :], srow[:])

            # masked max (valid positions only) for a stable softmax shift
            mx = rowp.tile([1, 1], F32, tag="mx")
            nc.vector.reduce_max(mx[:], srow[:], axis=AX.X)
            nmx = rowp.tile([1, 1], F32, tag="nmx")
            nc.vector.tensor_scalar_mul(nmx[:], mx[:], -1.0)

            # softmax on [1, T]; sum folded into the Exp via accum_out
            ew = rowp.tile([1, T], F32, tag="ew", bufs=2)
            sm = rowp.tile([1, 1], F32, tag="sm")
            nc.scalar.activation(
                ew[:], srow[:], AF.Exp, bias=nmx[:], accum_out=sm[:]
            )
            # unnormalized-exp transpose via one DMA: wcol[p, n] = ew[p*16+n];
            # the 1/sum scale is folded into the expectation epilogue
            wcol = rowp.tile([128, 16], F32, name=f"wcol{b}", tag=f"wcol{b}")
            nc.sync.dma_start(wcol[:], ew[:])
            wcol_h = rowp.tile([128, 16], F16, name=f"wcolh{b}", tag=f"wcolh{b}")
            nc.vector.tensor_copy(wcol_h[:], wcol[:])

            rc = rowp.tile([1, 1], F32, name=f"rc{b}", tag=f"rc{b}")
            nc.vector.reciprocal(rc[:], sm[:])
            # normalized weights overwrite srow's slot (srow dead after Exp)
            wgt_row = srow
            nc.vector.tensor_scalar(wgt_row[:], ew[:], rc[:], None, op0=ALU.mult)
            nc.sync.dma_start(weights_o[b:b + 1, :], wgt_row[:])
            return wcol_h, rc

        def tail_pe(b, wcol_h, rc):
            """expectation: expect[b,:] = rc * sum_n wcol[:,n].T @ xb[:,b,n,:]"""
            pe_t = pvec.tile([1, 512], F32, tag="pv")
            for n in range(16):
                nc.tensor.matmul(
                    pe_t[:], lhsT=wcol_h[:, n:n + 1], rhs=xb[:, b, n, :],
                    start=(n == 0), stop=(n == 15),
                )
            erow = rowp.tile([1, D], F32, tag="erow", bufs=2)
            nc.vector.tensor_scalar(erow[:], pe_t[:], rc[:], None, op0=ALU.mult)
            nc.sync.dma_start(expect_o[b:b + 1, :], erow[:])

        # repeat>1 re-runs the body for steady-state HW timing (outputs are
        # overwritten identically each time). All heads emitted before all
        # tails so the PE stream never stalls on a softmax chain.
        for rep in range(repeat):
            srows = {}
            for b in range(BL):
                srows[b] = batch_head(b)
                if rep == 0 and b == 0:
                    mask_prep()
            if rep == 0:
                prefetch_masks()
            wcols = {}
            for b in range(BL):
                wcols[b] = tail_pre(b, srows[b])
            for b in range(BL):
                tail_pe(b, *wcols[b])

    nc.compile()
    return nc


def _host_prep(w, u, v, query, lengths):
    """Host-side prep of the small replicated operands (layout + dtype)."""
    # wt_sb[p, dc, e] = w[e, dc*128+p]
    wt = np.ascontiguousarray(
        w.T.reshape(4, 128, D).transpose(1, 0, 2)
    ).astype(np.float16)
    # ub[b, e] = sum_q query[b, q] * u[e, q]
    ub = (query.astype(np.float64) @ u.T.astype(np.float64)).astype(np.float32)
    # vt[p, ec] = v[ec*128+p]
    vt = np.ascontiguousarray(v[:, 0].reshape(4, 128).T).astype(np.float16)
    lenf = lengths.astype(np.float32).reshape(B, 1)
    return wt, ub, vt, lenf


def make_in_maps(output_data, query, lengths, w, u, v):
    wt, ub, vt, lenf = _host_prep(w, u, v, query, lengths)
    in_maps = []
    for c in range(NCORES):
        sl = slice(c * BL, (c + 1) * BL)
        ub_s = np.ascontiguousarray(
            ub[sl].reshape(BL, 4, 128).transpose(2, 1, 0)
        )
        in_maps.append({
            "x": np.ascontiguousarray(output_data[sl]),
            "wt": wt,
            "ubt": ub_s,
            "vt": vt,
            "lc": np.ascontiguousarray(lenf[sl]),
        })
    return in_maps


def kernel(output_data, query, lengths, w, u, v):
    from concourse import bass_utils

    if "nc" not in _CACHE:
        _CACHE["nc"] = _build_nc()
    nc = _CACHE["nc"]

    in_maps = make_in_maps(output_data, query, lengths, w, u, v)
    res = bass_utils.run_bass_kernel_spmd(
        nc, in_maps, core_ids=list(range(NCORES))
    )
    score = np.concatenate(
        [res.results[c]["score_o"] for c in range(NCORES)], axis=0
    ).reshape(B, T, 1)
    weights = np.concatenate(
        [res.results[c]["weights_o"] for c in range(NCORES)], axis=0
    ).reshape(B, T, 1)
    expectation = np.concatenate(
        [res.results[c]["expect_o"] for c in range(NCORES)], axis=0
    )
    return score, weights, expectation
